# revision 8
# baseline (speedup 1.0000x reference)
"""Trainium2 Bass kernel for nn_MACE_openmm (periodic MACE edge-energy surrogate).

Strategy
--------
energy = sum over candidate pairs (s,i,j) within cutoff of
         W2^T silu(W1^T (bessel(r) * fcut(r)) + b1) + b2

Host-side (O(N) per shift): prune the 27*N candidate sender rows (s,i) by
distance of the shifted center c = 10*pos_i - shift_s to the AABB of the
scaled positions; rows further than R_MAX cannot produce any edge.  Survivors
(~1700 of 13824) are distributed over 8 cores, 256 row slots each (2 tiles of
128 rows x 512 receivers).

Device-side per tile [128 rows x 512 receivers]:
  r2      one K=5 matmul (lhsT rows [cx,cy,cz,|c|^2,1] vs rhs rows
          [-2px,-2py,-2pz,1,|p|^2]); tile 0 carries 64 extra contraction
          rows adding BIG at the true self-pair elements (s=0, j==i).
  clamp   r2s = clip(r2, EPS, CLAMPHI), CLAMPHI just under R_MAX^2
  ln/exp  r = exp(0.5 ln r2s); rinv' = exp(-0.5 ln r2s + ln(0.5*pref))
  fcut    cos = Sin(-pi/5 r + pi/2); t1 = max(cos + 1 - eps, 0);
          g = t1 * rinv'  -> exact 0 for every out-of-cutoff pair
  sins    w = k*(pi/5)*r, k=1..8, via static replication matmuls;
          z = (w+pi) mod 2pi on DVE; sin(z-pi) on ACT
  mlp1    h = blockdiag(W1)^T (sin * g_rep) via matmuls
  silu    Silu(h + b1) with fused accum_out row-sum.  Because g == 0
          exactly off-cutoff, masked pairs contribute exactly silu(b1);
          the host subtracts that in closed form and applies W2 to the
          128-lane accumulator (lane = 16*pair_row_in_block + channel).
"""
import sys
import contextlib

sys.path.insert(0, '/opt/trn_rl_repo')

import numpy as np

import concourse.bass as bass
import concourse.tile as tile
from concourse import bacc, mybir
from concourse.bass_utils import run_bass_kernel_spmd
from concourse.tile import ScopedClock
from concourse.vector_clock import VectorClock
from concourse.tile_sem_assignment import N_PROCS

f32 = mybir.dt.float32
AF = mybir.ActivationFunctionType
ALU = mybir.AluOpType

R_MAX = 5.0
SELF_SCALE = 10.0
N = 512
NCORES = 8
ROWS_PER_CORE = 256
T_TILES = 2
BIG = 1e9
EPS = 1e-7
CLAMPHI = 24.9999
FC_EPS = 2e-7
PREF = float(np.sqrt(2.0 / R_MAX))
PI = float(np.pi)
LOG_HALF_PREF = float(np.log(0.5 * PREF))
TOTAL_SLOTS = NCORES * ROWS_PER_CORE * N


def _rep_pattern(parity, scale_by_k):
    """[128,128] lhsT for the replication matmuls.

    For a 32-aligned source slice of r rows, out[8*lb + k, f] =
    val(k) * r_slice[16*parity + lb, f], lb in [0,16), k in [0,8).
    The 32-row pattern is tiled 4x so any 32-aligned K-slice works.
    """
    pat = np.zeros((32, 128), dtype=np.float32)
    for lb in range(16):
        for k in range(8):
            val = (k + 1) * (PI / R_MAX) if scale_by_k else 1.0
            pat[16 * parity + lb, 8 * lb + k] = val
    return np.tile(pat, (4, 1))


def _build_nc():
    nc = bacc.Bacc()

    def register_const(value):
        t = nc.alloc_sbuf_tensor(f"const-float32-{value}", [128, 1], f32)
        nc.gpsimd.memset(t.ap(), value)
        nc.const_aps.aps[(f32, value)] = t.ap()

    register_const(float(-PI))
    register_const(float(PI / 2))
    register_const(LOG_HALF_PREF)
    nc.all_engine_barrier()

    # ---- dram I/O ----
    lhsT_all = nc.dram_tensor("lhsT_all", [T_TILES, 69, 128], f32, kind="ExternalInput")
    rhsP = nc.dram_tensor("rhsP", [69, 512], f32, kind="ExternalInput")
    w1bd = nc.dram_tensor("w1bd", [128, 128], f32, kind="ExternalInput")
    b1vec = nc.dram_tensor("b1vec", [128, 1], f32, kind="ExternalInput")
    o_accs = nc.dram_tensor("o_accs", [128, T_TILES * 8], f32, kind="ExternalOutput")
    o_kc = nc.dram_tensor("o_kc", [128, T_TILES], f32, kind="ExternalOutput")

    repS = nc.inline_tensor(np.stack([_rep_pattern(0, True), _rep_pattern(1, True)]),
                            name="repS")    # [2,128,128] in DRAM
    repG = nc.inline_tensor(np.stack([_rep_pattern(0, False), _rep_pattern(1, False)]),
                            name="repG")

    with tile.TileContext(nc) as tc:
        with contextlib.ExitStack() as ctx:
            const = ctx.enter_context(tc.tile_pool(name="const", bufs=1))
            sb = ctx.enter_context(tc.tile_pool(name="sb", bufs=2))
            scr = ctx.enter_context(tc.tile_pool(name="scr", bufs=2))
            outp = ctx.enter_context(tc.tile_pool(name="outp", bufs=1))
            ps_r2 = ctx.enter_context(tc.tile_pool(name="ps_r2", bufs=1, space="PSUM"))
            ps_w = ctx.enter_context(tc.tile_pool(name="ps_w", bufs=1, space="PSUM"))
            ps_g = ctx.enter_context(tc.tile_pool(name="ps_g", bufs=1, space="PSUM"))
            ps_h = ctx.enter_context(tc.tile_pool(name="ps_h", bufs=1, space="PSUM"))

            lhsT_sb = []
            for t in range(T_TILES):
                lt = const.tile([69, 128], f32, name=f"lhsT{t}_sb")
                nc.sync.dma_start(out=lt[:, :], in_=lhsT_all[t, :, :])
                lhsT_sb.append(lt)
            rhs_sb = const.tile([69, 512], f32)
            nc.sync.dma_start(out=rhs_sb[:, :], in_=rhsP[:, :])
            w1_sb = const.tile([128, 128], f32)
            nc.sync.dma_start(out=w1_sb[:, :], in_=w1bd[:, :])
            b1_sb = const.tile([128, 1], f32)
            nc.sync.dma_start(out=b1_sb[:, :], in_=b1vec[:, :])
            repS_sb, repG_sb = [], []
            for par in range(2):
                tS = const.tile([128, 128], f32, name=f"repS{par}_sb")
                nc.sync.dma_start(out=tS[:, :], in_=repS[par, :, :])
                repS_sb.append(tS)
                tG = const.tile([128, 128], f32, name=f"repG{par}_sb")
                nc.sync.dma_start(out=tG[:, :], in_=repG[par, :, :])
                repG_sb.append(tG)

            accs_sb = outp.tile([128, T_TILES * 8], f32)
            kc_sb = outp.tile([128, T_TILES], f32)

            for tt in range(T_TILES):
                K = 69 if tt == 0 else 5
                r2_ps = ps_r2.tile([128, 512], f32, tag="r2")
                nc.tensor.matmul(r2_ps[:, :], lhsT_sb[tt][:K, :], rhs_sb[:K, :],
                                 start=True, stop=True)
                r2s = sb.tile([128, 512], f32, tag="r2s")
                nc.vector.tensor_scalar(r2s[:, :], r2_ps[:, :], EPS, CLAMPHI,
                                        op0=ALU.max, op1=ALU.min)
                keepscr = scr.tile([128, 512], f32, tag="keepscr")
                nc.vector.tensor_scalar(keepscr[:, :], r2s[:, :], CLAMPHI, 1.0,
                                        op0=ALU.is_lt, op1=ALU.mult,
                                        accum_out=kc_sb[:, tt:tt + 1])
                lnt = sb.tile([128, 512], f32, tag="lnt")
                nc.scalar.activation(lnt[:, :], r2s[:, :], AF.Ln)
                r_sb = sb.tile([128, 512], f32, tag="r_sb")
                nc.scalar.activation(r_sb[:, :], lnt[:, :], AF.Exp, scale=0.5)
                rinv = sb.tile([128, 512], f32, tag="rinv")
                nc.scalar.activation(rinv[:, :], lnt[:, :], AF.Exp, scale=-0.5,
                                     bias=LOG_HALF_PREF)
                cost = sb.tile([128, 512], f32, tag="cost")
                nc.scalar.activation(cost[:, :], r_sb[:, :], AF.Sin,
                                     scale=float(-PI / R_MAX), bias=float(PI / 2))
                t1 = sb.tile([128, 512], f32, tag="t1")
                nc.vector.tensor_scalar(t1[:, :], cost[:, :], float(1.0 - FC_EPS), 0.0,
                                        op0=ALU.add, op1=ALU.max)
                g_sb = sb.tile([128, 512], f32, tag="g_sb")
                nc.vector.tensor_tensor(g_sb[:, :], t1[:, :], rinv[:, :], ALU.mult)

                for q in range(4):
                    w_ps = ps_w.tile([128, 1024], f32, tag="w_ps")
                    g_ps = ps_g.tile([128, 1024], f32, tag="g_ps")
                    for h in range(2):
                        st = 2 * q + h
                        a32 = 32 * (st // 2)
                        par = st % 2
                        nc.tensor.matmul(w_ps[:, 512 * h:512 * h + 512],
                                         repS_sb[par][a32:a32 + 32, :],
                                         r_sb[a32:a32 + 32, :],
                                         start=True, stop=True,
                                         tile_position=(a32, 0))
                        nc.tensor.matmul(g_ps[:, 512 * h:512 * h + 512],
                                         repG_sb[par][a32:a32 + 32, :],
                                         g_sb[a32:a32 + 32, :],
                                         start=True, stop=True,
                                         tile_position=(a32, 0))
                    # range reduction: n = round(w/2pi) via i32 convert,
                    # z = w - 2pi*n in [-pi, pi]
                    n_i32 = sb.tile([128, 1024], mybir.dt.int32, tag="n_i32")
                    nc.vector.tensor_scalar(n_i32[:, :], w_ps[:, :],
                                            float(1.0 / (2 * PI)), None, op0=ALU.mult)
                    zfix = sb.tile([128, 1024], f32, tag="zfix")
                    nc.gpsimd.tensor_scalar(zfix[:, :], n_i32[:, :],
                                            float(-2 * PI), None, op0=ALU.mult)
                    z_sb = sb.tile([128, 1024], f32, tag="z_sb")
                    nc.vector.tensor_tensor(z_sb[:, :], w_ps[:, :], zfix[:, :],
                                            ALU.add)
                    sin_sb = sb.tile([128, 1024], f32, tag="sin_sb")
                    nc.scalar.activation(sin_sb[:, :], z_sb[:, :], AF.Sin)
                    rhs1 = sb.tile([128, 1024], f32, tag="rhs1")
                    nc.vector.tensor_tensor(rhs1[:, :], sin_sb[:, :], g_ps[:, :],
                                            ALU.mult)
                    for h in range(2):
                        h_ps = ps_h.tile([128, 1024], f32, tag="h_ps")
                        for x in range(2):
                            nc.tensor.matmul(
                                h_ps[:, 512 * x:512 * x + 512],
                                w1_sb[64 * x:64 * x + 64, :],
                                rhs1[64 * x:64 * x + 64, 512 * h:512 * h + 512],
                                start=True, stop=True)
                        hs = scr.tile([128, 1024], f32, tag="hs")
                        slot = tt * 8 + 2 * q + h
                        nc.scalar.activation(hs[:, :], h_ps[:, :], AF.Silu,
                                             bias=b1_sb[:, :],
                                             accum_out=accs_sb[:, slot:slot + 1])

            nc.sync.dma_start(out=o_accs[:, :], in_=accs_sb[:, :])
            nc.sync.dma_start(out=o_kc[:, :], in_=kc_sb[:, :])
    nc.finalize()
    return nc


_NC_CACHE = None


def _get_nc():
    global _NC_CACHE
    if _NC_CACHE is None:
        _NC_CACHE = _build_nc()
    return _NC_CACHE


def _host_prep(positions, boxVectors):
    fnp = np.float32
    pos = (np.asarray(positions, dtype=fnp) * SELF_SCALE).astype(fnp)
    cell = (np.asarray(boxVectors, dtype=fnp) * SELF_SCALE).astype(fnp)
    a = np.arange(-1, 2)
    S = np.stack(np.meshgrid(a, a, a, indexing="ij"), axis=-1).reshape(27, 3)
    shift_vecs = (S.astype(fnp) @ cell).astype(fnp)

    lo = pos.min(axis=0)
    hi = pos.max(axis=0)
    others = []
    for s in range(27):
        if s == 13:
            continue
        c = pos - shift_vecs[s]
        cl = np.clip(c, lo, hi)
        d2 = ((c - cl) ** 2).sum(axis=1)
        for i in np.where(d2 <= R_MAX * R_MAX)[0]:
            others.append(c[i])
    n_other = len(others)
    cap_other = (ROWS_PER_CORE - 64) * NCORES
    assert n_other <= cap_other, f"row overflow: {n_other} > {cap_other}"
    opc = -(-n_other // NCORES) if n_other else 0

    rhs_base = np.zeros((69, 512), dtype=fnp)
    rhs_base[0:3, :] = -2.0 * pos.T
    rhs_base[3, :] = 1.0
    rhs_base[4, :] = (pos * pos).sum(axis=1)

    in_maps = []
    for m in range(NCORES):
        C = np.full((ROWS_PER_CORE, 3), 1e4, dtype=fnp)
        C[0:64] = pos[64 * m:64 * m + 64]        # s=(0,0,0) self rows
        sl = others[m * opc:(m + 1) * opc]
        if sl:
            C[64:64 + len(sl)] = np.array(sl, dtype=fnp)
        lhsT = np.zeros((T_TILES, 69, 128), dtype=fnp)
        for t in range(T_TILES):
            Ct = C[128 * t:128 * t + 128]
            lhsT[t, 0:3, :] = Ct.T
            lhsT[t, 3, :] = (Ct * Ct).sum(axis=1)
            lhsT[t, 4, :] = 1.0
        lhsT[0, 5:69, 0:64] = BIG * np.eye(64, dtype=fnp)
        rhs = rhs_base.copy()
        cols = 64 * m + np.arange(64)
        rhs[5 + np.arange(64), cols] = 1.0
        in_maps.append({"lhsT_all": lhsT, "rhsP": rhs})
    return in_maps


def _host_weights(W1, b1):
    fnp = np.float32
    W1 = np.asarray(W1, dtype=fnp)
    b1 = np.asarray(b1, dtype=fnp)
    bd = np.zeros((64, 128), dtype=fnp)
    for b in range(8):
        bd[8 * b:8 * b + 8, 16 * b:16 * b + 16] = W1
    w1full = np.concatenate([bd, bd], axis=0)
    b1vec = np.tile(b1, 8).reshape(128, 1).astype(fnp)
    return w1full, b1vec


def kernel(positions, boxVectors, W1, b1, W2, b2):
    nc = _get_nc()
    in_maps = _host_prep(positions, boxVectors)
    w1full, b1vec = _host_weights(W1, b1)
    for im in in_maps:
        im["w1bd"] = w1full
        im["b1vec"] = b1vec
    res = run_bass_kernel_spmd(nc, in_maps, core_ids=list(range(NCORES)))

    W2v = np.asarray(W2, dtype=np.float64)[:, 0]
    b1v = np.asarray(b1, dtype=np.float64)
    b2v = float(np.asarray(b2)[0])
    w2lane = np.tile(W2v, 8)
    silu_bg = float((b1v / (1.0 + np.exp(-b1v))) @ W2v)

    total = 0.0
    nkeep = 0.0
    for m in range(NCORES):
        accs = res.results[m]["o_accs"].astype(np.float64)
        kc = res.results[m]["o_kc"].astype(np.float64)
        total += float(w2lane @ accs.sum(axis=1))
        nkeep += float(kc.sum())
    n_masked = TOTAL_SLOTS - nkeep
    total -= n_masked * silu_bg
    total += b2v * nkeep
    return np.float32(total)


# revision 11
# speedup vs baseline: 1.4051x; 1.4051x over previous
"""Trainium2 Bass kernel for nn_MACE_openmm (periodic MACE edge-energy surrogate).

Strategy
--------
energy = sum over candidate pairs (s,i,j) within cutoff of
         W2^T silu(W1^T (bessel(r) * fcut(r)) + b1) + b2

Host-side (O(N) per shift): prune the 27*N candidate sender rows (s,i) by
distance of the shifted center c = 10*pos_i - shift_s to the AABB of the
scaled positions; rows further than R_MAX cannot produce any edge.  Survivors
(~1700 of 13824) are distributed over 8 cores, 256 row slots each (2 tiles of
128 rows x 512 receivers).

Device-side per tile [128 rows x 512 receivers]:
  r2      one K=5 matmul (lhsT rows [cx,cy,cz,|c|^2,1] vs rhs rows
          [-2px,-2py,-2pz,1,|p|^2]); tile 0 carries 64 extra contraction
          rows adding BIG at the true self-pair elements (s=0, j==i).
  clamp   r2s = clip(r2, EPS, CLAMPHI), CLAMPHI just under R_MAX^2
  ln/exp  r = exp(0.5 ln r2s); rinv' = exp(-0.5 ln r2s + ln(0.5*pref))
  fcut    cos = Sin(-pi/5 r + pi/2); t1 = max(cos + 1 - eps, 0);
          g = t1 * rinv'  -> exact 0 for every out-of-cutoff pair
  sins    w = k*(pi/5)*r, k=1..8, via static replication matmuls;
          z = (w+pi) mod 2pi on DVE; sin(z-pi) on ACT
  mlp1    h = blockdiag(W1)^T (sin * g_rep) via matmuls
  silu    Silu(h + b1) with fused accum_out row-sum.  Because g == 0
          exactly off-cutoff, masked pairs contribute exactly silu(b1);
          the host subtracts that in closed form and applies W2 to the
          128-lane accumulator (lane = 16*pair_row_in_block + channel).
"""
import sys
import contextlib

sys.path.insert(0, '/opt/trn_rl_repo')

import numpy as np

import concourse.bass as bass
import concourse.tile as tile
from concourse import bacc, mybir
from concourse.bass_utils import run_bass_kernel_spmd
from concourse.tile import ScopedClock
from concourse.vector_clock import VectorClock
from concourse.tile_sem_assignment import N_PROCS

f32 = mybir.dt.float32
AF = mybir.ActivationFunctionType
ALU = mybir.AluOpType

R_MAX = 5.0
SELF_SCALE = 10.0
N = 512
NCORES = 8
ROWS_PER_CORE = 256
T_TILES = 2
BIG = 1e9
EPS = 1e-7
CLAMPHI = 24.9999
FC_EPS = 2e-7
PREF = float(np.sqrt(2.0 / R_MAX))
PI = float(np.pi)
LOG_HALF_PREF = float(np.log(0.5 * PREF))
TOTAL_SLOTS = NCORES * ROWS_PER_CORE * N


def _rep_pattern(parity, scale_by_k):
    """[128,128] lhsT for the replication matmuls.

    For a 32-aligned source slice of r rows, out[8*lb + k, f] =
    val(k) * r_slice[16*parity + lb, f], lb in [0,16), k in [0,8).
    The 32-row pattern is tiled 4x so any 32-aligned K-slice works.
    """
    pat = np.zeros((32, 128), dtype=np.float32)
    for lb in range(16):
        for k in range(8):
            val = (k + 1) * (PI / R_MAX) if scale_by_k else 1.0
            pat[16 * parity + lb, 8 * lb + k] = val
    return np.tile(pat, (4, 1))


def _build_nc():
    nc = bacc.Bacc()

    def register_const(value):
        t = nc.alloc_sbuf_tensor(f"const-float32-{value}", [128, 1], f32)
        nc.gpsimd.memset(t.ap(), value)
        nc.const_aps.aps[(f32, value)] = t.ap()

    register_const(float(-PI))
    register_const(float(PI / 2))
    register_const(LOG_HALF_PREF)
    nc.all_engine_barrier()

    # ---- dram I/O ----
    lhsT_all = nc.dram_tensor("lhsT_all", [T_TILES, 69, 128], f32, kind="ExternalInput")
    rhsP = nc.dram_tensor("rhsP", [69, 512], f32, kind="ExternalInput")
    w1bd = nc.dram_tensor("w1bd", [128, 128], f32, kind="ExternalInput")
    b1vec = nc.dram_tensor("b1vec", [128, 1], f32, kind="ExternalInput")
    o_accs = nc.dram_tensor("o_accs", [128, T_TILES * 8], f32, kind="ExternalOutput")
    o_kc = nc.dram_tensor("o_kc", [128, T_TILES], f32, kind="ExternalOutput")

    repS = nc.inline_tensor(np.stack([_rep_pattern(0, True), _rep_pattern(1, True)]),
                            name="repS")    # [2,128,128] in DRAM
    repG = nc.inline_tensor(np.stack([_rep_pattern(0, False), _rep_pattern(1, False)]),
                            name="repG")
    negI = nc.inline_tensor((-2 * PI) * np.eye(128, dtype=np.float32), name="negI")

    with tile.TileContext(nc) as tc:
        with contextlib.ExitStack() as ctx:
            const = ctx.enter_context(tc.tile_pool(name="const", bufs=1))
            sb = ctx.enter_context(tc.tile_pool(name="sb", bufs=2))
            scr = ctx.enter_context(tc.tile_pool(name="scr", bufs=2))
            outp = ctx.enter_context(tc.tile_pool(name="outp", bufs=1))
            ps_r2 = ctx.enter_context(tc.tile_pool(name="ps_r2", bufs=1, space="PSUM"))
            ps_w = ctx.enter_context(tc.tile_pool(name="ps_w", bufs=1, space="PSUM"))
            ps_g = ctx.enter_context(tc.tile_pool(name="ps_g", bufs=1, space="PSUM"))
            ps_h = ctx.enter_context(tc.tile_pool(name="ps_h", bufs=1, space="PSUM"))

            lhsT_sb = []
            for t in range(T_TILES):
                lt = const.tile([69, 128], f32, name=f"lhsT{t}_sb")
                nc.sync.dma_start(out=lt[:, :], in_=lhsT_all[t, :, :])
                lhsT_sb.append(lt)
            rhs_sb = const.tile([69, 512], f32)
            nc.sync.dma_start(out=rhs_sb[:, :], in_=rhsP[:, :])
            w1_sb = const.tile([128, 128], f32)
            nc.sync.dma_start(out=w1_sb[:, :], in_=w1bd[:, :])
            b1_sb = const.tile([128, 1], f32)
            nc.sync.dma_start(out=b1_sb[:, :], in_=b1vec[:, :])
            repS_sb, repG_sb = [], []
            for par in range(2):
                tS = const.tile([128, 128], f32, name=f"repS{par}_sb")
                nc.sync.dma_start(out=tS[:, :], in_=repS[par, :, :])
                repS_sb.append(tS)
                tG = const.tile([128, 128], f32, name=f"repG{par}_sb")
                nc.sync.dma_start(out=tG[:, :], in_=repG[par, :, :])
                repG_sb.append(tG)
            negI_sb = const.tile([128, 128], f32)
            nc.sync.dma_start(out=negI_sb[:, :], in_=negI[:, :])

            accs_sb = outp.tile([128, T_TILES * 8], f32)
            kc_sb = outp.tile([128, T_TILES], f32)

            for tt in range(T_TILES):
                K = 69 if tt == 0 else 5
                r2_ps = ps_r2.tile([128, 512], f32, tag="r2")
                nc.tensor.matmul(r2_ps[:, :], lhsT_sb[tt][:K, :], rhs_sb[:K, :],
                                 start=True, stop=True)
                r2s = sb.tile([128, 512], f32, tag="r2s")
                nc.vector.tensor_scalar(r2s[:, :], r2_ps[:, :], EPS, CLAMPHI,
                                        op0=ALU.max, op1=ALU.min)
                keepscr = scr.tile([128, 512], f32, tag="keepscr")
                nc.vector.tensor_scalar(keepscr[:, :], r2s[:, :], CLAMPHI, 1.0,
                                        op0=ALU.is_lt, op1=ALU.mult,
                                        accum_out=kc_sb[:, tt:tt + 1])
                lnt = sb.tile([128, 512], f32, tag="lnt")
                nc.scalar.activation(lnt[:, :], r2s[:, :], AF.Ln)
                r_sb = sb.tile([128, 512], f32, tag="r_sb")
                nc.scalar.activation(r_sb[:, :], lnt[:, :], AF.Exp, scale=0.5)
                rinv = sb.tile([128, 512], f32, tag="rinv")
                nc.scalar.activation(rinv[:, :], lnt[:, :], AF.Exp, scale=-0.5,
                                     bias=LOG_HALF_PREF)
                cost = sb.tile([128, 512], f32, tag="cost")
                nc.scalar.activation(cost[:, :], r_sb[:, :], AF.Sin,
                                     scale=float(-PI / R_MAX), bias=float(PI / 2))
                t1 = sb.tile([128, 512], f32, tag="t1")
                nc.vector.tensor_scalar(t1[:, :], cost[:, :], float(1.0 - FC_EPS), 0.0,
                                        op0=ALU.add, op1=ALU.max)
                g_sb = sb.tile([128, 512], f32, tag="g_sb")
                nc.vector.tensor_tensor(g_sb[:, :], t1[:, :], rinv[:, :], ALU.mult)

                for q in range(4):
                    w_ps = ps_w.tile([128, 1024], f32, tag="w_ps")
                    g_ps = ps_g.tile([128, 1024], f32, tag="g_ps")
                    for h in range(2):
                        st = 2 * q + h
                        a32 = 32 * (st // 2)
                        par = st % 2
                        nc.tensor.matmul(w_ps[:, 512 * h:512 * h + 512],
                                         repS_sb[par][a32:a32 + 32, :],
                                         r_sb[a32:a32 + 32, :],
                                         start=True, stop=True,
                                         tile_position=(a32, 0))
                        nc.tensor.matmul(g_ps[:, 512 * h:512 * h + 512],
                                         repG_sb[par][a32:a32 + 32, :],
                                         g_sb[a32:a32 + 32, :],
                                         start=True, stop=True,
                                         tile_position=(a32, 0))
                    # range reduction: n = round(w/2pi) via i32 convert;
                    # w -= 2pi*n done on the PE by accumulating (-2pi*I) @ nf
                    # into w_ps, leaving z = w - 2pi*n in [-pi, pi] in PSUM.
                    n_i32 = sb.tile([128, 1024], mybir.dt.int32, tag="n_i32")
                    nc.vector.tensor_scalar(n_i32[:, :], w_ps[:, :],
                                            float(1.0 / (2 * PI)), None, op0=ALU.mult)
                    nf = sb.tile([128, 1024], f32, tag="nf")
                    nc.vector.tensor_scalar(nf[:, :], n_i32[:, :], 1.0, None,
                                            op0=ALU.mult)
                    for h in range(2):
                        nc.tensor.matmul(w_ps[:, 512 * h:512 * h + 512],
                                         negI_sb[:, :],
                                         nf[:, 512 * h:512 * h + 512],
                                         start=False, stop=True,
                                         skip_group_check=True)
                    sin_sb = sb.tile([128, 1024], f32, tag="sin_sb")
                    nc.scalar.activation(sin_sb[:, :], w_ps[:, :], AF.Sin)
                    rhs1 = sb.tile([128, 1024], f32, tag="rhs1")
                    nc.vector.tensor_tensor(rhs1[:, :], sin_sb[:, :], g_ps[:, :],
                                            ALU.mult)
                    for h in range(2):
                        h_ps = ps_h.tile([128, 1024], f32, tag="h_ps")
                        for x in range(2):
                            nc.tensor.matmul(
                                h_ps[:, 512 * x:512 * x + 512],
                                w1_sb[64 * x:64 * x + 64, :],
                                rhs1[64 * x:64 * x + 64, 512 * h:512 * h + 512],
                                start=True, stop=True)
                        hs = scr.tile([128, 1024], f32, tag="hs")
                        slot = tt * 8 + 2 * q + h
                        nc.scalar.activation(hs[:, :], h_ps[:, :], AF.Silu,
                                             bias=b1_sb[:, :],
                                             accum_out=accs_sb[:, slot:slot + 1])

            nc.sync.dma_start(out=o_accs[:, :], in_=accs_sb[:, :])
            nc.sync.dma_start(out=o_kc[:, :], in_=kc_sb[:, :])
    nc.finalize()
    return nc


_NC_CACHE = None


def _get_nc():
    global _NC_CACHE
    if _NC_CACHE is None:
        _NC_CACHE = _build_nc()
    return _NC_CACHE


def _host_prep(positions, boxVectors):
    fnp = np.float32
    pos = (np.asarray(positions, dtype=fnp) * SELF_SCALE).astype(fnp)
    cell = (np.asarray(boxVectors, dtype=fnp) * SELF_SCALE).astype(fnp)
    a = np.arange(-1, 2)
    S = np.stack(np.meshgrid(a, a, a, indexing="ij"), axis=-1).reshape(27, 3)
    shift_vecs = (S.astype(fnp) @ cell).astype(fnp)

    lo = pos.min(axis=0)
    hi = pos.max(axis=0)
    others = []
    for s in range(27):
        if s == 13:
            continue
        c = pos - shift_vecs[s]
        cl = np.clip(c, lo, hi)
        d2 = ((c - cl) ** 2).sum(axis=1)
        for i in np.where(d2 <= R_MAX * R_MAX)[0]:
            others.append(c[i])
    n_other = len(others)
    cap_other = (ROWS_PER_CORE - 64) * NCORES
    assert n_other <= cap_other, f"row overflow: {n_other} > {cap_other}"
    opc = -(-n_other // NCORES) if n_other else 0

    rhs_base = np.zeros((69, 512), dtype=fnp)
    rhs_base[0:3, :] = -2.0 * pos.T
    rhs_base[3, :] = 1.0
    rhs_base[4, :] = (pos * pos).sum(axis=1)

    in_maps = []
    for m in range(NCORES):
        C = np.full((ROWS_PER_CORE, 3), 1e4, dtype=fnp)
        C[0:64] = pos[64 * m:64 * m + 64]        # s=(0,0,0) self rows
        sl = others[m * opc:(m + 1) * opc]
        if sl:
            C[64:64 + len(sl)] = np.array(sl, dtype=fnp)
        lhsT = np.zeros((T_TILES, 69, 128), dtype=fnp)
        for t in range(T_TILES):
            Ct = C[128 * t:128 * t + 128]
            lhsT[t, 0:3, :] = Ct.T
            lhsT[t, 3, :] = (Ct * Ct).sum(axis=1)
            lhsT[t, 4, :] = 1.0
        lhsT[0, 5:69, 0:64] = BIG * np.eye(64, dtype=fnp)
        rhs = rhs_base.copy()
        cols = 64 * m + np.arange(64)
        rhs[5 + np.arange(64), cols] = 1.0
        in_maps.append({"lhsT_all": lhsT, "rhsP": rhs})
    return in_maps


def _host_weights(W1, b1):
    fnp = np.float32
    W1 = np.asarray(W1, dtype=fnp)
    b1 = np.asarray(b1, dtype=fnp)
    bd = np.zeros((64, 128), dtype=fnp)
    for b in range(8):
        bd[8 * b:8 * b + 8, 16 * b:16 * b + 16] = W1
    w1full = np.concatenate([bd, bd], axis=0)
    b1vec = np.tile(b1, 8).reshape(128, 1).astype(fnp)
    return w1full, b1vec


def kernel(positions, boxVectors, W1, b1, W2, b2):
    nc = _get_nc()
    in_maps = _host_prep(positions, boxVectors)
    w1full, b1vec = _host_weights(W1, b1)
    for im in in_maps:
        im["w1bd"] = w1full
        im["b1vec"] = b1vec
    res = run_bass_kernel_spmd(nc, in_maps, core_ids=list(range(NCORES)))

    W2v = np.asarray(W2, dtype=np.float64)[:, 0]
    b1v = np.asarray(b1, dtype=np.float64)
    b2v = float(np.asarray(b2)[0])
    w2lane = np.tile(W2v, 8)
    silu_bg = float((b1v / (1.0 + np.exp(-b1v))) @ W2v)

    total = 0.0
    nkeep = 0.0
    for m in range(NCORES):
        accs = res.results[m]["o_accs"].astype(np.float64)
        kc = res.results[m]["o_kc"].astype(np.float64)
        total += float(w2lane @ accs.sum(axis=1))
        nkeep += float(kc.sum())
    n_masked = TOTAL_SLOTS - nkeep
    total -= n_masked * silu_bg
    total += b2v * nkeep
    return np.float32(total)


# revision 17
# speedup vs baseline: 1.7649x; 1.2560x over previous
"""Trainium2 Bass kernel for nn_MACE_openmm (periodic MACE edge-energy surrogate).

Strategy
--------
energy = sum over candidate pairs (s,i,j) within cutoff of
         W2^T silu(W1^T (bessel(r) * fcut(r)) + b1) + b2

Host-side (O(N) per shift): prune the 27*N candidate sender rows (s,i) by
distance of the shifted center c = 10*pos_i - shift_s to the AABB of the
scaled positions; rows further than R_MAX cannot produce any edge.  Survivors
(~1700 of 13824) are distributed over 8 cores, 256 row slots each (2 tiles of
128 rows x 512 receivers).

Device-side per tile [128 rows x 512 receivers]:
  r2      one K=5 matmul (lhsT rows [cx,cy,cz,|c|^2,1] vs rhs rows
          [-2px,-2py,-2pz,1,|p|^2]); tile 0 carries 64 extra contraction
          rows adding BIG at the true self-pair elements (s=0, j==i).
  clamp   r2s = clip(r2, EPS, CLAMPHI), CLAMPHI just under R_MAX^2
  ln/exp  r = exp(0.5 ln r2s); rinv' = exp(-0.5 ln r2s + ln(0.5*pref))
  fcut    cos = Sin(-pi/5 r + pi/2); t1 = max(cos + 1 - eps, 0);
          g = t1 * rinv'  -> exact 0 for every out-of-cutoff pair
  sins    w = k*(pi/5)*r, k=1..8, via static replication matmuls;
          z = (w+pi) mod 2pi on DVE; sin(z-pi) on ACT
  mlp1    h = blockdiag(W1)^T (sin * g_rep) via matmuls
  silu    Silu(h + b1) with fused accum_out row-sum.  Because g == 0
          exactly off-cutoff, masked pairs contribute exactly silu(b1);
          the host subtracts that in closed form and applies W2 to the
          128-lane accumulator (lane = 16*pair_row_in_block + channel).
"""
import sys
import contextlib

sys.path.insert(0, '/opt/trn_rl_repo')

import numpy as np

import concourse.bass as bass
import concourse.tile as tile
from concourse import bacc, mybir
from concourse.bass_utils import run_bass_kernel_spmd
from concourse.tile import ScopedClock
from concourse.vector_clock import VectorClock
from concourse.tile_sem_assignment import N_PROCS

f32 = mybir.dt.float32
AF = mybir.ActivationFunctionType
ALU = mybir.AluOpType

R_MAX = 5.0
SELF_SCALE = 10.0
N = 512
NCORES = 8
ROWS_PER_CORE = 256
T_TILES = 2
BIG = 1e9
EPS = 1e-7
CLAMPHI = 24.9999
FC_EPS = 2e-7
PREF = float(np.sqrt(2.0 / R_MAX))
PI = float(np.pi)
LOG_HALF_PREF = float(np.log(0.5 * PREF))
TOTAL_SLOTS = NCORES * ROWS_PER_CORE * N


def _rep_pattern(parity, scale_by_k):
    """[128,128] lhsT for the replication matmuls.

    For a 32-aligned source slice of r rows, out[8*lb + k, f] =
    val(k) * r_slice[16*parity + lb, f], lb in [0,16), k in [0,8).
    The 32-row pattern is tiled 4x so any 32-aligned K-slice works.
    """
    pat = np.zeros((32, 128), dtype=np.float32)
    for lb in range(16):
        for k in range(8):
            val = (k + 1) * (PI / R_MAX) if scale_by_k else 1.0
            pat[16 * parity + lb, 8 * lb + k] = val
    return np.tile(pat, (4, 1))


def _build_nc():
    nc = bacc.Bacc()

    def register_const(value):
        t = nc.alloc_sbuf_tensor(f"const-float32-{value}", [128, 1], f32)
        nc.gpsimd.memset(t.ap(), value)
        nc.const_aps.aps[(f32, value)] = t.ap()

    register_const(float(-PI))
    register_const(float(PI / 2))
    register_const(LOG_HALF_PREF)
    nc.all_engine_barrier()

    # ---- dram I/O ----
    lhsT_all = nc.dram_tensor("lhsT_all", [T_TILES, 69, 128], f32, kind="ExternalInput")
    rhsP = nc.dram_tensor("rhsP", [69, 512], f32, kind="ExternalInput")
    w1bd = nc.dram_tensor("w1bd", [128, 128], f32, kind="ExternalInput")
    b1vec = nc.dram_tensor("b1vec", [128, 1], f32, kind="ExternalInput")
    o_accs = nc.dram_tensor("o_accs", [128, T_TILES * 8], f32, kind="ExternalOutput")
    o_kc = nc.dram_tensor("o_kc", [128, T_TILES], f32, kind="ExternalOutput")

    repS = nc.inline_tensor(np.stack([_rep_pattern(0, True), _rep_pattern(1, True)]),
                            name="repS")    # [2,128,128] in DRAM
    repG = nc.inline_tensor(np.stack([_rep_pattern(0, False), _rep_pattern(1, False)]),
                            name="repG")
    negI = nc.inline_tensor((-2 * PI) * np.eye(128, dtype=np.float32), name="negI")

    with tile.TileContext(nc) as tc:
        with contextlib.ExitStack() as ctx:
            const = ctx.enter_context(tc.tile_pool(name="const", bufs=1))
            sb = ctx.enter_context(tc.tile_pool(name="sb", bufs=2))
            scr = ctx.enter_context(tc.tile_pool(name="scr", bufs=2))
            outp = ctx.enter_context(tc.tile_pool(name="outp", bufs=1))
            ps_w = ctx.enter_context(tc.tile_pool(name="ps_w", bufs=2, space="PSUM"))
            ps_h = ctx.enter_context(tc.tile_pool(name="ps_h", bufs=2, space="PSUM"))

            lhsT_sb = []
            for t in range(T_TILES):
                lt = const.tile([69, 128], f32, name=f"lhsT{t}_sb")
                nc.sync.dma_start(out=lt[:, :], in_=lhsT_all[t, :, :])
                lhsT_sb.append(lt)
            rhs_sb = const.tile([69, 512], f32)
            nc.sync.dma_start(out=rhs_sb[:, :], in_=rhsP[:, :])
            w1_sb = const.tile([128, 128], f32)
            nc.sync.dma_start(out=w1_sb[:, :], in_=w1bd[:, :])
            b1_sb = const.tile([128, 1], f32)
            nc.sync.dma_start(out=b1_sb[:, :], in_=b1vec[:, :])
            repS_sb, repG_sb = [], []
            for par in range(2):
                tS = const.tile([128, 128], f32, name=f"repS{par}_sb")
                nc.sync.dma_start(out=tS[:, :], in_=repS[par, :, :])
                repS_sb.append(tS)
                tG = const.tile([128, 128], f32, name=f"repG{par}_sb")
                nc.sync.dma_start(out=tG[:, :], in_=repG[par, :, :])
                repG_sb.append(tG)
            negI_sb = const.tile([128, 128], f32)
            nc.sync.dma_start(out=negI_sb[:, :], in_=negI[:, :])

            accs_sb = outp.tile([128, T_TILES * 8], f32)
            kc_sb = outp.tile([128, T_TILES], f32)

            for tt in range(T_TILES):
                K = 69 if tt == 0 else 5
                r2f = ps_w.tile([128, 1024], f32, tag="w_ps")
                r2_ps = r2f[:, 0:512]
                nc.tensor.matmul(r2_ps[:, :], lhsT_sb[tt][:K, :], rhs_sb[:K, :],
                                 start=True, stop=True)
                r2s = sb.tile([128, 512], f32, tag="r2s")
                nc.vector.tensor_scalar(r2s[:, :], r2_ps[:, :], EPS, CLAMPHI,
                                        op0=ALU.max, op1=ALU.min)
                keepscr = scr.tile([128, 512], f32, tag="keepscr")
                nc.vector.tensor_scalar(keepscr[:, :], r2s[:, :], CLAMPHI, 1.0,
                                        op0=ALU.is_lt, op1=ALU.mult,
                                        accum_out=kc_sb[:, tt:tt + 1])
                lnt = sb.tile([128, 512], f32, tag="lnt")
                nc.scalar.activation(lnt[:, :], r2s[:, :], AF.Ln)
                r_sb = sb.tile([128, 512], f32, tag="r_sb")
                nc.scalar.activation(r_sb[:, :], lnt[:, :], AF.Exp, scale=0.5)
                rinv = sb.tile([128, 512], f32, tag="rinv")
                nc.scalar.activation(rinv[:, :], lnt[:, :], AF.Exp, scale=-0.5,
                                     bias=LOG_HALF_PREF)
                cost = sb.tile([128, 512], f32, tag="cost")
                nc.scalar.activation(cost[:, :], r_sb[:, :], AF.Sin,
                                     scale=float(-PI / R_MAX), bias=float(PI / 2))
                t1 = sb.tile([128, 512], f32, tag="t1")
                nc.vector.tensor_scalar(t1[:, :], cost[:, :], float(1.0 - FC_EPS), 0.0,
                                        op0=ALU.add, op1=ALU.max)
                g_sb = sb.tile([128, 512], f32, tag="g_sb")
                nc.vector.tensor_tensor(g_sb[:, :], t1[:, :], rinv[:, :], ALU.mult)

                for q in range(4):
                    w_ps = ps_w.tile([128, 1024], f32, tag="w_ps")
                    g_rep = sb.tile([128, 1024], f32, tag="g_rep")
                    for h in range(2):
                        st = 2 * q + h
                        a32 = 32 * (st // 2)
                        par = st % 2
                        nc.tensor.matmul(w_ps[:, 512 * h:512 * h + 512],
                                         repS_sb[par][a32:a32 + 32, :],
                                         r_sb[a32:a32 + 32, :],
                                         start=True, stop=True,
                                         tile_position=(a32, 0))
                        # replicate g rows 16*st..16*st+16 into 8 harmonic
                        # rows each: 8 DMAs with partition-strided dest
                        src = g_sb[16 * st:16 * st + 16, :]
                        dstv = g_rep[:, 512 * h:512 * h + 512].rearrange(
                            "(a b) f -> b a f", b=8)   # [8, 16, 512]
                        for k in range(8):
                            nc.sync.dma_start(out=dstv[k], in_=src)
                    # range reduction: n = round(w/2pi) via i32 convert;
                    # w -= 2pi*n done on the PE by accumulating (-2pi*I) @ nf
                    # into w_ps, leaving z = w - 2pi*n in [-pi, pi] in PSUM.
                    n_i32 = sb.tile([128, 1024], mybir.dt.int32, tag="n_i32")
                    nc.vector.tensor_scalar(n_i32[:, :], w_ps[:, :],
                                            float(1.0 / (2 * PI)), None, op0=ALU.mult)
                    nf = sb.tile([128, 1024], f32, tag="nf")
                    nc.vector.tensor_scalar(nf[:, :], n_i32[:, :], 1.0, None,
                                            op0=ALU.mult)
                    for h in range(2):
                        nc.tensor.matmul(w_ps[:, 512 * h:512 * h + 512],
                                         negI_sb[:, :],
                                         nf[:, 512 * h:512 * h + 512],
                                         start=False, stop=True,
                                         skip_group_check=True)
                    sin_sb = sb.tile([128, 1024], f32, tag="sin_sb")
                    nc.scalar.activation(sin_sb[:, :], w_ps[:, :], AF.Sin)
                    rhs1 = sb.tile([128, 1024], f32, tag="rhs1")
                    nc.vector.tensor_tensor(rhs1[:, :], sin_sb[:, :], g_rep[:, :],
                                            ALU.mult)
                    for h in range(2):
                        h_ps = ps_h.tile([128, 1024], f32, tag="h_ps")
                        for x in range(2):
                            nc.tensor.matmul(
                                h_ps[:, 512 * x:512 * x + 512],
                                w1_sb[64 * x:64 * x + 64, :],
                                rhs1[64 * x:64 * x + 64, 512 * h:512 * h + 512],
                                start=True, stop=True)
                        hs = scr.tile([128, 1024], f32, tag="hs")
                        slot = tt * 8 + 2 * q + h
                        nc.scalar.activation(hs[:, :], h_ps[:, :], AF.Silu,
                                             bias=b1_sb[:, :],
                                             accum_out=accs_sb[:, slot:slot + 1])

            nc.sync.dma_start(out=o_accs[:, :], in_=accs_sb[:, :])
            nc.sync.dma_start(out=o_kc[:, :], in_=kc_sb[:, :])
    nc.finalize()
    return nc


_NC_CACHE = None


def _get_nc():
    global _NC_CACHE
    if _NC_CACHE is None:
        _NC_CACHE = _build_nc()
    return _NC_CACHE


def _host_prep(positions, boxVectors):
    fnp = np.float32
    pos = (np.asarray(positions, dtype=fnp) * SELF_SCALE).astype(fnp)
    cell = (np.asarray(boxVectors, dtype=fnp) * SELF_SCALE).astype(fnp)
    a = np.arange(-1, 2)
    S = np.stack(np.meshgrid(a, a, a, indexing="ij"), axis=-1).reshape(27, 3)
    shift_vecs = (S.astype(fnp) @ cell).astype(fnp)

    lo = pos.min(axis=0)
    hi = pos.max(axis=0)
    others = []
    for s in range(27):
        if s == 13:
            continue
        c = pos - shift_vecs[s]
        cl = np.clip(c, lo, hi)
        d2 = ((c - cl) ** 2).sum(axis=1)
        for i in np.where(d2 <= R_MAX * R_MAX)[0]:
            others.append(c[i])
    n_other = len(others)
    cap_other = (ROWS_PER_CORE - 64) * NCORES
    assert n_other <= cap_other, f"row overflow: {n_other} > {cap_other}"
    opc = -(-n_other // NCORES) if n_other else 0

    rhs_base = np.zeros((69, 512), dtype=fnp)
    rhs_base[0:3, :] = -2.0 * pos.T
    rhs_base[3, :] = 1.0
    rhs_base[4, :] = (pos * pos).sum(axis=1)

    in_maps = []
    for m in range(NCORES):
        C = np.full((ROWS_PER_CORE, 3), 1e4, dtype=fnp)
        C[0:64] = pos[64 * m:64 * m + 64]        # s=(0,0,0) self rows
        sl = others[m * opc:(m + 1) * opc]
        if sl:
            C[64:64 + len(sl)] = np.array(sl, dtype=fnp)
        lhsT = np.zeros((T_TILES, 69, 128), dtype=fnp)
        for t in range(T_TILES):
            Ct = C[128 * t:128 * t + 128]
            lhsT[t, 0:3, :] = Ct.T
            lhsT[t, 3, :] = (Ct * Ct).sum(axis=1)
            lhsT[t, 4, :] = 1.0
        lhsT[0, 5:69, 0:64] = BIG * np.eye(64, dtype=fnp)
        rhs = rhs_base.copy()
        cols = 64 * m + np.arange(64)
        rhs[5 + np.arange(64), cols] = 1.0
        in_maps.append({"lhsT_all": lhsT, "rhsP": rhs})
    return in_maps


def _host_weights(W1, b1):
    fnp = np.float32
    W1 = np.asarray(W1, dtype=fnp)
    b1 = np.asarray(b1, dtype=fnp)
    bd = np.zeros((64, 128), dtype=fnp)
    for b in range(8):
        bd[8 * b:8 * b + 8, 16 * b:16 * b + 16] = W1
    w1full = np.concatenate([bd, bd], axis=0)
    b1vec = np.tile(b1, 8).reshape(128, 1).astype(fnp)
    return w1full, b1vec


def kernel(positions, boxVectors, W1, b1, W2, b2):
    nc = _get_nc()
    in_maps = _host_prep(positions, boxVectors)
    w1full, b1vec = _host_weights(W1, b1)
    for im in in_maps:
        im["w1bd"] = w1full
        im["b1vec"] = b1vec
    res = run_bass_kernel_spmd(nc, in_maps, core_ids=list(range(NCORES)))

    W2v = np.asarray(W2, dtype=np.float64)[:, 0]
    b1v = np.asarray(b1, dtype=np.float64)
    b2v = float(np.asarray(b2)[0])
    w2lane = np.tile(W2v, 8)
    silu_bg = float((b1v / (1.0 + np.exp(-b1v))) @ W2v)

    total = 0.0
    nkeep = 0.0
    for m in range(NCORES):
        accs = res.results[m]["o_accs"].astype(np.float64)
        kc = res.results[m]["o_kc"].astype(np.float64)
        total += float(w2lane @ accs.sum(axis=1))
        nkeep += float(kc.sum())
    n_masked = TOTAL_SLOTS - nkeep
    total -= n_masked * silu_bg
    total += b2v * nkeep
    return np.float32(total)


# revision 20
# speedup vs baseline: 1.7756x; 1.0061x over previous
"""Trainium2 Bass kernel for nn_MACE_openmm (periodic MACE edge-energy surrogate).

Strategy
--------
energy = sum over candidate pairs (s,i,j) within cutoff of
         W2^T silu(W1^T (bessel(r) * fcut(r)) + b1) + b2

Host-side (O(N) per shift): prune the 27*N candidate sender rows (s,i) by
distance of the shifted center c = 10*pos_i - shift_s to the AABB of the
scaled positions; rows further than R_MAX cannot produce any edge.  Survivors
(~1700 of 13824) are distributed over 8 cores, 256 row slots each (2 tiles of
128 rows x 512 receivers).

Device-side per tile [128 rows x 512 receivers]:
  r2      one K=5 matmul (lhsT rows [cx,cy,cz,|c|^2,1] vs rhs rows
          [-2px,-2py,-2pz,1,|p|^2]); tile 0 carries 64 extra contraction
          rows adding BIG at the true self-pair elements (s=0, j==i).
  clamp   r2s = clip(r2, EPS, CLAMPHI), CLAMPHI just under R_MAX^2
  ln/exp  r = exp(0.5 ln r2s); rinv' = exp(-0.5 ln r2s + ln(0.5*pref))
  fcut    cos = Sin(-pi/5 r + pi/2); t1 = max(cos + 1 - eps, 0);
          g = t1 * rinv'  -> exact 0 for every out-of-cutoff pair
  sins    w = k*(pi/5)*r, k=1..8, via static replication matmuls;
          z = (w+pi) mod 2pi on DVE; sin(z-pi) on ACT
  mlp1    h = blockdiag(W1)^T (sin * g_rep) via matmuls
  silu    Silu(h + b1) with fused accum_out row-sum.  Because g == 0
          exactly off-cutoff, masked pairs contribute exactly silu(b1);
          the host subtracts that in closed form and applies W2 to the
          128-lane accumulator (lane = 16*pair_row_in_block + channel).
"""
import sys
import contextlib

sys.path.insert(0, '/opt/trn_rl_repo')

import numpy as np

import concourse.bass as bass
import concourse.tile as tile
from concourse import bacc, mybir
from concourse.bass_utils import run_bass_kernel_spmd
from concourse.tile import ScopedClock
from concourse.vector_clock import VectorClock
from concourse.tile_sem_assignment import N_PROCS

f32 = mybir.dt.float32
AF = mybir.ActivationFunctionType
ALU = mybir.AluOpType

R_MAX = 5.0
SELF_SCALE = 10.0
N = 512
NCORES = 8
ROWS_PER_CORE = 256
T_TILES = 2
BIG = 1e9
EPS = 1e-7
CLAMPHI = 24.9999
FC_EPS = 2e-7
PREF = float(np.sqrt(2.0 / R_MAX))
PI = float(np.pi)
LOG_HALF_PREF = float(np.log(0.5 * PREF))
TOTAL_SLOTS = NCORES * ROWS_PER_CORE * N


def _rep_pattern(parity, scale_by_k):
    """[128,128] lhsT for the replication matmuls.

    For a 32-aligned source slice of r rows, out[8*lb + k, f] =
    val(k) * r_slice[16*parity + lb, f], lb in [0,16), k in [0,8).
    The 32-row pattern is tiled 4x so any 32-aligned K-slice works.
    """
    pat = np.zeros((32, 128), dtype=np.float32)
    for lb in range(16):
        for k in range(8):
            val = (k + 1) * (PI / R_MAX) if scale_by_k else 1.0
            pat[16 * parity + lb, 8 * lb + k] = val
    return np.tile(pat, (4, 1))


def _build_nc():
    nc = bacc.Bacc()

    def register_const(value):
        t = nc.alloc_sbuf_tensor(f"const-float32-{value}", [128, 1], f32)
        nc.gpsimd.memset(t.ap(), value)
        nc.const_aps.aps[(f32, value)] = t.ap()

    register_const(float(-PI))
    register_const(float(PI / 2))
    register_const(LOG_HALF_PREF)
    nc.all_engine_barrier()

    # ---- dram I/O ----
    lhsT_all = nc.dram_tensor("lhsT_all", [T_TILES, 69, 128], f32, kind="ExternalInput")
    rhsP = nc.dram_tensor("rhsP", [69, 512], f32, kind="ExternalInput")
    w1bd = nc.dram_tensor("w1bd", [128, 128], f32, kind="ExternalInput")
    b1vec = nc.dram_tensor("b1vec", [128, 1], f32, kind="ExternalInput")
    o_accs = nc.dram_tensor("o_accs", [128, T_TILES * 8], f32, kind="ExternalOutput")
    o_kc = nc.dram_tensor("o_kc", [128, T_TILES], f32, kind="ExternalOutput")

    repS = nc.inline_tensor(np.stack([_rep_pattern(0, True), _rep_pattern(1, True)]),
                            name="repS")    # [2,128,128] in DRAM
    repG = nc.inline_tensor(np.stack([_rep_pattern(0, False), _rep_pattern(1, False)]),
                            name="repG")
    negI = nc.inline_tensor((-2 * PI) * np.eye(128, dtype=np.float32), name="negI")

    with tile.TileContext(nc) as tc:
        with contextlib.ExitStack() as ctx:
            const = ctx.enter_context(tc.tile_pool(name="const", bufs=1))
            sb = ctx.enter_context(tc.tile_pool(name="sb", bufs=2))
            scr = ctx.enter_context(tc.tile_pool(name="scr", bufs=2))
            outp = ctx.enter_context(tc.tile_pool(name="outp", bufs=1))
            ps_w = ctx.enter_context(tc.tile_pool(name="ps_w", bufs=3, space="PSUM"))
            ps_h = ctx.enter_context(tc.tile_pool(name="ps_h", bufs=1, space="PSUM"))

            lhsT_sb = []
            for t in range(T_TILES):
                lt = const.tile([69, 128], f32, name=f"lhsT{t}_sb")
                nc.sync.dma_start(out=lt[:, :], in_=lhsT_all[t, :, :])
                lhsT_sb.append(lt)
            rhs_sb = const.tile([69, 512], f32)
            nc.sync.dma_start(out=rhs_sb[:, :], in_=rhsP[:, :])
            w1_sb = const.tile([128, 128], f32)
            nc.sync.dma_start(out=w1_sb[:, :], in_=w1bd[:, :])
            b1_sb = const.tile([128, 1], f32)
            nc.sync.dma_start(out=b1_sb[:, :], in_=b1vec[:, :])
            repS_sb, repG_sb = [], []
            for par in range(2):
                tS = const.tile([128, 128], f32, name=f"repS{par}_sb")
                nc.sync.dma_start(out=tS[:, :], in_=repS[par, :, :])
                repS_sb.append(tS)
                tG = const.tile([128, 128], f32, name=f"repG{par}_sb")
                nc.sync.dma_start(out=tG[:, :], in_=repG[par, :, :])
                repG_sb.append(tG)
            negI_sb = const.tile([128, 128], f32)
            nc.sync.dma_start(out=negI_sb[:, :], in_=negI[:, :])

            accs_sb = outp.tile([128, T_TILES * 8], f32)
            kc_sb = outp.tile([128, T_TILES], f32)

            for tt in range(T_TILES):
                K = 69 if tt == 0 else 5
                r2f = ps_w.tile([128, 1024], f32, tag="w_ps")
                r2_ps = r2f[:, 0:512]
                nc.tensor.matmul(r2_ps[:, :], lhsT_sb[tt][:K, :], rhs_sb[:K, :],
                                 start=True, stop=True)
                r2s = sb.tile([128, 512], f32, tag="r2s")
                nc.vector.tensor_scalar(r2s[:, :], r2_ps[:, :], EPS, CLAMPHI,
                                        op0=ALU.max, op1=ALU.min)
                keepscr = scr.tile([128, 512], f32, tag="keepscr")
                nc.vector.tensor_scalar(keepscr[:, :], r2s[:, :], CLAMPHI, 1.0,
                                        op0=ALU.is_lt, op1=ALU.mult,
                                        accum_out=kc_sb[:, tt:tt + 1])
                lnt = sb.tile([128, 512], f32, tag="lnt")
                nc.scalar.activation(lnt[:, :], r2s[:, :], AF.Ln)
                r_sb = sb.tile([128, 512], f32, tag="r_sb")
                nc.scalar.activation(r_sb[:, :], lnt[:, :], AF.Exp, scale=0.5)
                rinv = sb.tile([128, 512], f32, tag="rinv")
                nc.scalar.activation(rinv[:, :], lnt[:, :], AF.Exp, scale=-0.5,
                                     bias=LOG_HALF_PREF)
                cost = sb.tile([128, 512], f32, tag="cost")
                nc.scalar.activation(cost[:, :], r_sb[:, :], AF.Sin,
                                     scale=float(-PI / R_MAX), bias=float(PI / 2))
                t1 = sb.tile([128, 512], f32, tag="t1")
                nc.vector.tensor_scalar(t1[:, :], cost[:, :], float(1.0 - FC_EPS), 0.0,
                                        op0=ALU.add, op1=ALU.max)
                g_sb = sb.tile([128, 512], f32, tag="g_sb")
                nc.vector.tensor_tensor(g_sb[:, :], t1[:, :], rinv[:, :], ALU.mult)

                for q in range(4):
                    w_ps = ps_w.tile([128, 1024], f32, tag="w_ps")
                    g_rep = sb.tile([128, 1024], f32, tag="g_rep")
                    for h in range(2):
                        st = 2 * q + h
                        a32 = 32 * (st // 2)
                        par = st % 2
                        nc.tensor.matmul(w_ps[:, 512 * h:512 * h + 512],
                                         repS_sb[par][a32:a32 + 32, :],
                                         r_sb[a32:a32 + 32, :],
                                         start=True, stop=True,
                                         tile_position=(a32, 0))
                        # replicate g rows 16*st..16*st+16 into 8 harmonic
                        # rows each: 8 DMAs with partition-strided dest,
                        # dispatched from the otherwise-idle gpsimd queues
                        src = g_sb[16 * st:16 * st + 16, :]
                        dstv = g_rep[:, 512 * h:512 * h + 512].rearrange(
                            "(a b) f -> b a f", b=8)   # [8, 16, 512]
                        for k in range(8):
                            nc.gpsimd.dma_start(out=dstv[k], in_=src)
                    # range reduction: n = round(w/2pi) via i32 convert,
                    # z = w - 2pi*n in [-pi, pi]
                    n_i32 = sb.tile([128, 1024], mybir.dt.int32, tag="n_i32")
                    nc.vector.tensor_scalar(n_i32[:, :], w_ps[:, :],
                                            float(1.0 / (2 * PI)), None, op0=ALU.mult)
                    nf = sb.tile([128, 1024], f32, tag="nf")
                    nc.vector.tensor_scalar(nf[:, :], n_i32[:, :],
                                            float(-2 * PI), None, op0=ALU.mult)
                    z_sb = sb.tile([128, 1024], f32, tag="z_sb")
                    nc.vector.tensor_tensor(z_sb[:, :], w_ps[:, :], nf[:, :],
                                            ALU.add)
                    sin_sb = sb.tile([128, 1024], f32, tag="sin_sb")
                    nc.scalar.activation(sin_sb[:, :], z_sb[:, :], AF.Sin)
                    rhs1 = sb.tile([128, 1024], f32, tag="rhs1")
                    nc.vector.tensor_tensor(rhs1[:, :], sin_sb[:, :], g_rep[:, :],
                                            ALU.mult)
                    for h in range(2):
                        h_ps = ps_h.tile([128, 1024], f32, tag="h_ps")
                        for x in range(2):
                            nc.tensor.matmul(
                                h_ps[:, 512 * x:512 * x + 512],
                                w1_sb[64 * x:64 * x + 64, :],
                                rhs1[64 * x:64 * x + 64, 512 * h:512 * h + 512],
                                start=True, stop=True)
                        hs = scr.tile([128, 1024], f32, tag="hs")
                        slot = tt * 8 + 2 * q + h
                        nc.scalar.activation(hs[:, :], h_ps[:, :], AF.Silu,
                                             bias=b1_sb[:, :],
                                             accum_out=accs_sb[:, slot:slot + 1])

            nc.sync.dma_start(out=o_accs[:, :], in_=accs_sb[:, :])
            nc.sync.dma_start(out=o_kc[:, :], in_=kc_sb[:, :])
    nc.finalize()
    return nc


_NC_CACHE = None


def _get_nc():
    global _NC_CACHE
    if _NC_CACHE is None:
        _NC_CACHE = _build_nc()
    return _NC_CACHE


def _host_prep(positions, boxVectors):
    fnp = np.float32
    pos = (np.asarray(positions, dtype=fnp) * SELF_SCALE).astype(fnp)
    cell = (np.asarray(boxVectors, dtype=fnp) * SELF_SCALE).astype(fnp)
    a = np.arange(-1, 2)
    S = np.stack(np.meshgrid(a, a, a, indexing="ij"), axis=-1).reshape(27, 3)
    shift_vecs = (S.astype(fnp) @ cell).astype(fnp)

    lo = pos.min(axis=0)
    hi = pos.max(axis=0)
    others = []
    for s in range(27):
        if s == 13:
            continue
        c = pos - shift_vecs[s]
        cl = np.clip(c, lo, hi)
        d2 = ((c - cl) ** 2).sum(axis=1)
        for i in np.where(d2 <= R_MAX * R_MAX)[0]:
            others.append(c[i])
    n_other = len(others)
    cap_other = (ROWS_PER_CORE - 64) * NCORES
    assert n_other <= cap_other, f"row overflow: {n_other} > {cap_other}"
    opc = -(-n_other // NCORES) if n_other else 0

    rhs_base = np.zeros((69, 512), dtype=fnp)
    rhs_base[0:3, :] = -2.0 * pos.T
    rhs_base[3, :] = 1.0
    rhs_base[4, :] = (pos * pos).sum(axis=1)

    in_maps = []
    for m in range(NCORES):
        C = np.full((ROWS_PER_CORE, 3), 1e4, dtype=fnp)
        C[0:64] = pos[64 * m:64 * m + 64]        # s=(0,0,0) self rows
        sl = others[m * opc:(m + 1) * opc]
        if sl:
            C[64:64 + len(sl)] = np.array(sl, dtype=fnp)
        lhsT = np.zeros((T_TILES, 69, 128), dtype=fnp)
        for t in range(T_TILES):
            Ct = C[128 * t:128 * t + 128]
            lhsT[t, 0:3, :] = Ct.T
            lhsT[t, 3, :] = (Ct * Ct).sum(axis=1)
            lhsT[t, 4, :] = 1.0
        lhsT[0, 5:69, 0:64] = BIG * np.eye(64, dtype=fnp)
        rhs = rhs_base.copy()
        cols = 64 * m + np.arange(64)
        rhs[5 + np.arange(64), cols] = 1.0
        in_maps.append({"lhsT_all": lhsT, "rhsP": rhs})
    return in_maps


def _host_weights(W1, b1):
    fnp = np.float32
    W1 = np.asarray(W1, dtype=fnp)
    b1 = np.asarray(b1, dtype=fnp)
    bd = np.zeros((64, 128), dtype=fnp)
    for b in range(8):
        bd[8 * b:8 * b + 8, 16 * b:16 * b + 16] = W1
    w1full = np.concatenate([bd, bd], axis=0)
    b1vec = np.tile(b1, 8).reshape(128, 1).astype(fnp)
    return w1full, b1vec


def kernel(positions, boxVectors, W1, b1, W2, b2):
    nc = _get_nc()
    in_maps = _host_prep(positions, boxVectors)
    w1full, b1vec = _host_weights(W1, b1)
    for im in in_maps:
        im["w1bd"] = w1full
        im["b1vec"] = b1vec
    res = run_bass_kernel_spmd(nc, in_maps, core_ids=list(range(NCORES)))

    W2v = np.asarray(W2, dtype=np.float64)[:, 0]
    b1v = np.asarray(b1, dtype=np.float64)
    b2v = float(np.asarray(b2)[0])
    w2lane = np.tile(W2v, 8)
    silu_bg = float((b1v / (1.0 + np.exp(-b1v))) @ W2v)

    total = 0.0
    nkeep = 0.0
    for m in range(NCORES):
        accs = res.results[m]["o_accs"].astype(np.float64)
        kc = res.results[m]["o_kc"].astype(np.float64)
        total += float(w2lane @ accs.sum(axis=1))
        nkeep += float(kc.sum())
    n_masked = TOTAL_SLOTS - nkeep
    total -= n_masked * silu_bg
    total += b2v * nkeep
    return np.float32(total)
